# revision 1
# baseline (speedup 1.0000x reference)
"""ExpertBank Trainium2 kernel: LayerNorm -> per-expert [affine -> Linear(512,1024)
-> GELU(erf) -> Linear(1024,512)] for all 16 experts, expert-parallel over 8 cores.

Strategy per core (2 experts):
  - Host folds the LN affine into W1/b1 (exact algebra) and precomputes the
    per-token LN scale/shift, so the device only does:
      x_hat = x * scale + shift              (one DVE op per token tile)
      x_hatT = PE-transpose(x_hat)           (fp32, via identity matmul)
      hT    = gelu(W1c.T @ x_hatT + b1)      (fp32r matmuls, ACT gelu w/ bias)
      out   = hT.T-slices @ W2c + b2         (fp32r matmuls, DVE adds b2)
  - fp32r (tf32-like, ~12-bit mantissa) runs matmuls at 1 cycle/row: 4x faster
    than plain fp32, with fp32 PSUM accumulation.
  - Output [8192, 2, 512] per core; host concatenates expert axis.
"""
import numpy as np

import concourse.tile as tile
import concourse.mybir as mybir
from concourse import bacc
from concourse.bass import ds
from concourse.bass_utils import run_bass_kernel_spmd

F32 = mybir.dt.float32
F32R = mybir.dt.float32r

B, S, H, F, E = 4, 2048, 512, 1024, 16
N = B * S                 # 8192 tokens
NCORES = 8
E_LOC = E // NCORES       # 2 experts per core
EPS = 1e-5
TBLK = 512                # tokens per block
NBLK = N // TBLK          # 16
KH = H // 128             # 4 contraction chunks for GEMM1
KF = F // 128             # 8 contraction chunks for GEMM2
MF = F // 128             # 8 output chunks for GEMM1
MT = TBLK // 128          # 4 token subtiles per block

GELU = mybir.ActivationFunctionType.Gelu
ADD = mybir.AluOpType.add
MULT = mybir.AluOpType.mult

_COMPILED = None


def _build():
    nc = bacc.Bacc("TRN2", debug=False, enable_asserts=False,
                   target_bir_lowering=False)
    tok_d = nc.dram_tensor("tokens", [N, H], F32, kind="ExternalInput").ap()
    w1_d = nc.dram_tensor("w1", [E_LOC, KH, 128, F], F32R, kind="ExternalInput").ap()
    w2_d = nc.dram_tensor("w2", [E_LOC, KF, 128, H], F32R, kind="ExternalInput").ap()
    b1_d = nc.dram_tensor("b1c", [128, E_LOC * MF], F32, kind="ExternalInput").ap()
    b2_d = nc.dram_tensor("b2bc", [E_LOC, 128, H], F32, kind="ExternalInput").ap()
    scl_d = nc.dram_tensor("scl", [128, N // 128], F32, kind="ExternalInput").ap()
    shf_d = nc.dram_tensor("shf", [128, N // 128], F32, kind="ExternalInput").ap()
    id_d = nc.dram_tensor("ident", [128, 128], F32, kind="ExternalInput").ap()
    out_d = nc.dram_tensor("out", [N, E_LOC, H], F32, kind="ExternalOutput").ap()

    with tile.TileContext(nc) as tc:
        with tc.tile_pool(name="const", bufs=1) as cst, \
             tc.tile_pool(name="io", bufs=1) as io, \
             tc.tile_pool(name="ps", bufs=1, space="PSUM") as ps:
            # --- resident constants ---
            w1t = [[cst.tile_from(w1_d[e, k], name=f"w1_{e}_{k}")
                    for k in range(KH)] for e in range(E_LOC)]
            w2t = [[cst.tile_from(w2_d[e, k], name=f"w2_{e}_{k}")
                    for k in range(KF)] for e in range(E_LOC)]
            b1t = cst.tile_from(b1_d, name="b1t")
            b2t = [cst.tile_from(b2_d[e], name=f"b2_{e}") for e in range(E_LOC)]
            sclt = cst.tile_from(scl_d, name="sclt")
            shft = cst.tile_from(shf_d, name="shft")
            idt = cst.tile_from(id_d, name="idt")

            for b in range(NBLK):
                tok0 = b * TBLK
                # --- LN apply + transpose into x_hatT (f32r) ---
                xT = [io.tile([128, TBLK], F32R, name="xT", tag="xT", bufs=2 * KH)
                      for _ in range(KH)]
                for t in range(MT):
                    g = b * MT + t    # global token-tile index
                    x_t = io.tile([128, H], F32, name="x_t", tag="x_t", bufs=2 * MT)
                    nc.sync.dma_start(x_t, tok_d[ds(tok0 + t * 128, 128), :])
                    xh_t = io.tile([128, H], F32, name="xh_t", tag="xh_t",
                                   bufs=2 * MT)
                    nc.vector.tensor_scalar(xh_t, x_t, sclt[:, g:g + 1],
                                            shft[:, g:g + 1], MULT, ADD)
                    for k in range(KH):
                        pt = ps.tile([128, 128], F32, name="pt", tag="pt", bufs=3)
                        nc.tensor.transpose(pt, xh_t[:, ds(k * 128, 128)], idt)
                        nc.vector.tensor_copy(xT[k][:, ds(t * 128, 128)], pt)

                for e in range(E_LOC):
                    # --- GEMM1 + GELU: hT[mf] = gelu(W1c.T @ xT + b1) ---
                    hT = [io.tile([128, TBLK], F32R, name="hT", tag="hT",
                                  bufs=2 * MF) for _ in range(MF)]
                    for mf in range(MF):
                        pm1 = ps.tile([128, TBLK], F32, name="pm1", tag="pm1",
                                      bufs=2)
                        for k in range(KH):
                            nc.tensor.matmul(pm1, w1t[e][k][:, ds(mf * 128, 128)],
                                             xT[k], start=(k == 0),
                                             stop=(k == KH - 1))
                        nc.scalar.activation(hT[mf], pm1, GELU,
                                             bias=b1t[:, e * MF + mf:e * MF + mf + 1],
                                             scale=1.0)
                    # --- GEMM2 + b2: out[tok, :] = hT.T @ W2c + b2 ---
                    for mt in range(MT):
                        pm2 = ps.tile([128, H], F32, name="pm2", tag="pm2", bufs=2)
                        for k in range(KF):
                            nc.tensor.matmul(pm2, hT[k][:, ds(mt * 128, 128)],
                                             w2t[e][k], start=(k == 0),
                                             stop=(k == KF - 1))
                        o_t = io.tile([128, H], F32, name="o_t", tag="o_t", bufs=8)
                        nc.vector.tensor_tensor(o_t, pm2, b2t[e], ADD)
                        nc.sync.dma_start(
                            out_d[ds(tok0 + mt * 128, 128), e, :], o_t)
    nc.compile()
    return nc


def _get_compiled():
    global _COMPILED
    if _COMPILED is None:
        _COMPILED = _build()
    return _COMPILED


def _prepare_in_maps(tokens, ln_g, ln_b, W1, b1, W2, b2):
    x = np.ascontiguousarray(np.asarray(tokens, dtype=np.float32).reshape(N, H))
    # LN stats (float64 internally; matches fp32 reference to ~1e-7 rel)
    x64 = x.astype(np.float64)
    mu = x64.mean(axis=1)
    var = np.square(x64 - mu[:, None]).mean(axis=1)
    rstd = 1.0 / np.sqrt(var + EPS)
    scale = rstd.astype(np.float32)
    shift = (-mu * rstd).astype(np.float32)

    # Fold LN affine into W1/b1: (x_hat*g + b) @ W1 + b1 = x_hat @ (g*W1) + (b@W1 + b1)
    W1 = np.asarray(W1, dtype=np.float32)
    W2 = np.asarray(W2, dtype=np.float32)
    ln_g = np.asarray(ln_g, dtype=np.float32)
    ln_b = np.asarray(ln_b, dtype=np.float32)
    b1 = np.asarray(b1, dtype=np.float32)
    b2 = np.asarray(b2, dtype=np.float32)
    W1eff = (ln_g[:, :, None] * W1).astype(np.float32)
    b1eff = (np.einsum('eh,ehf->ef', ln_b.astype(np.float64),
                       W1.astype(np.float64)) + b1).astype(np.float32)

    scl = np.ascontiguousarray(scale.reshape(N // 128, 128).T)
    shf = np.ascontiguousarray(shift.reshape(N // 128, 128).T)
    ident = np.eye(128, dtype=np.float32)

    in_maps = []
    for c in range(NCORES):
        e0 = c * E_LOC
        sl = slice(e0, e0 + E_LOC)
        in_maps.append({
            "tokens": x,
            "w1": np.ascontiguousarray(W1eff[sl].reshape(E_LOC, KH, 128, F)),
            "w2": np.ascontiguousarray(W2[sl].reshape(E_LOC, KF, 128, H)),
            "b1c": np.ascontiguousarray(
                b1eff[sl].reshape(E_LOC, MF, 128).transpose(2, 0, 1)
                .reshape(128, E_LOC * MF)),
            "b2bc": np.ascontiguousarray(
                np.broadcast_to(b2[sl][:, None, :], (E_LOC, 128, H))),
            "scl": scl,
            "shf": shf,
            "ident": ident,
        })
    return in_maps


def _run(in_maps, trace=False, **kw):
    nc = _get_compiled()
    return run_bass_kernel_spmd(nc, in_maps, core_ids=list(range(NCORES)),
                                trace=trace, **kw)


def kernel(tokens, ln_g, ln_b, W1, b1, W2, b2):
    in_maps = _prepare_in_maps(tokens, ln_g, ln_b, W1, b1, W2, b2)
    res = _run(in_maps)
    parts = [res.results[c]["out"] for c in range(NCORES)]   # [N, E_LOC, H] each
    full = np.concatenate(parts, axis=1).reshape(B, S, E, H)
    return full.astype(np.float32)


# revision 4
# speedup vs baseline: 1.0622x; 1.0622x over previous
"""ExpertBank Trainium2 kernel: LayerNorm -> per-expert [affine -> Linear(512,1024)
-> GELU(erf) -> Linear(1024,512)] for all 16 experts, expert-parallel over 8 cores.

Strategy per core (2 experts):
  - Host folds the LN affine into W1/b1 (exact algebra) and precomputes the
    per-token LN scale/shift, so the device only does:
      x_hat = x * scale + shift              (one DVE op per token tile, -> f32r)
      x_hatT = PE-transpose(x_hat)           (f32r, via identity matmul)
      hT    = gelu(W1c.T @ x_hatT + b1)      (fp32r matmuls, ACT gelu w/ bias)
      out   = hT.T-slices @ W2c + b2         (fp32r matmuls, DVE adds b2)
  - fp32r (tf32-like, ~12-bit mantissa) runs matmuls at 1 cycle/row: 4x faster
    than plain fp32, with fp32 PSUM accumulation.
  - Emission is software-pipelined (next block's LN before this block's GEMMs)
    and token/const DMAs are queued before the bulk weight DMAs so the PE
    starts within a few us.
  - Output [8192, 2, 512] per core; host concatenates the expert axis.
"""
import numpy as np

import concourse.tile as tile
import concourse.mybir as mybir
from concourse import bacc
from concourse.bass import ds
from concourse.bass_utils import run_bass_kernel_spmd

F32 = mybir.dt.float32
F32R = mybir.dt.float32r

B, S, H, F, E = 4, 2048, 512, 1024, 16
N = B * S                 # 8192 tokens
NCORES = 8
E_LOC = E // NCORES       # 2 experts per core
EPS = 1e-5
TBLK = 512                # tokens per block
NBLK = N // TBLK          # 16
KH = H // 128             # 4 contraction chunks for GEMM1
KF = F // 128             # 8 contraction chunks for GEMM2
MF = F // 128             # 8 output chunks for GEMM1
MT = TBLK // 128          # 4 token subtiles per block

GELU = mybir.ActivationFunctionType.Gelu
ADD = mybir.AluOpType.add
MULT = mybir.AluOpType.mult

_COMPILED = None


def _build():
    nc = bacc.Bacc("TRN2", debug=False, enable_asserts=False,
                   target_bir_lowering=False)
    tok_d = nc.dram_tensor("tokens", [N, H], F32, kind="ExternalInput").ap()
    w1_d = nc.dram_tensor("w1", [E_LOC, KH, 128, F], F32R, kind="ExternalInput").ap()
    w2_d = nc.dram_tensor("w2", [E_LOC, KF, 128, H], F32R, kind="ExternalInput").ap()
    b1_d = nc.dram_tensor("b1c", [128, E_LOC * MF], F32, kind="ExternalInput").ap()
    b2_d = nc.dram_tensor("b2bc", [E_LOC, 128, H], F32, kind="ExternalInput").ap()
    scl_d = nc.dram_tensor("scl", [128, N // 128], F32, kind="ExternalInput").ap()
    shf_d = nc.dram_tensor("shf", [128, N // 128], F32, kind="ExternalInput").ap()
    id_d = nc.dram_tensor("ident", [128, 128], F32, kind="ExternalInput").ap()
    out_d = nc.dram_tensor("out", [N, E_LOC, H], F32, kind="ExternalOutput").ap()

    with tile.TileContext(nc) as tc:
        with tc.tile_pool(name="const", bufs=1) as cst, \
             tc.tile_pool(name="io", bufs=1) as io, \
             tc.tile_pool(name="ps", bufs=1, space="PSUM") as ps:
            # --- small resident constants first (cheap DMAs, unblock LN) ---
            b1t = cst.tile_from(b1_d, name="b1t")
            b2t = [cst.tile_from(b2_d[e], name=f"b2_{e}") for e in range(E_LOC)]
            sclt = cst.tile_from(scl_d, name="sclt")
            shft = cst.tile_from(shf_d, name="shft")
            idt = cst.tile_from(id_d, name="idt")

            def emit_ln(b):
                """DMA token tiles of block b and apply LN scale/shift (DVE)."""
                xh = []
                for t in range(MT):
                    g = b * MT + t    # global token-tile index
                    x_t = io.tile([128, H], F32, name="x_t", tag="x_t",
                                  bufs=3 * MT)
                    nc.sync.dma_start(x_t, tok_d[ds(b * TBLK + t * 128, 128), :])
                    xh_t = io.tile([128, H], F32, name="xh_t", tag="xh_t",
                                   bufs=3 * MT)
                    nc.vector.tensor_scalar(xh_t, x_t, sclt[:, g:g + 1],
                                            shft[:, g:g + 1], MULT, ADD)
                    xh.append(xh_t)
                return xh

            def emit_transposes(xh):
                """PE-transpose block's x_hat into x_hatT chunks (f32r)."""
                xT = [io.tile([128, TBLK], F32R, name="xT", tag="xT", bufs=2 * KH)
                      for _ in range(KH)]
                for t in range(MT):
                    for k in range(KH):
                        pt = ps.tile([128, 128], F32, name="pt", tag="pt", bufs=3)
                        nc.tensor.transpose(pt, xh[t][:, ds(k * 128, 128)], idt)
                        nc.vector.tensor_copy(xT[k][:, ds(t * 128, 128)], pt)
                return xT

            def emit_experts(b, xT):
                tok0 = b * TBLK
                for e in range(E_LOC):
                    # --- GEMM1 + GELU: hT[mf] = gelu(W1c.T @ xT + b1) ---
                    hT = [io.tile([128, TBLK], F32R, name="hT", tag="hT",
                                  bufs=2 * MF) for _ in range(MF)]
                    for mf in range(MF):
                        pm1 = ps.tile([128, TBLK], F32, name="pm1", tag="pm1",
                                      bufs=3)
                        for k in range(KH):
                            nc.tensor.matmul(pm1, w1t[e][k][:, ds(mf * 128, 128)],
                                             xT[k], start=(k == 0),
                                             stop=(k == KH - 1))
                        nc.scalar.activation(hT[mf], pm1, GELU,
                                             bias=b1t[:, e * MF + mf:e * MF + mf + 1],
                                             scale=1.0)
                    # --- GEMM2 + b2: out[tok, :] = hT.T @ W2c + b2 ---
                    for mt in range(MT):
                        pm2 = ps.tile([128, H], F32, name="pm2", tag="pm2", bufs=2)
                        for k in range(KF):
                            nc.tensor.matmul(pm2, hT[k][:, ds(mt * 128, 128)],
                                             w2t[e][k], start=(k == 0),
                                             stop=(k == KF - 1))
                        o_t = io.tile([128, H], F32, name="o_t", tag="o_t", bufs=8)
                        nc.vector.tensor_tensor(o_t, pm2, b2t[e], ADD)
                        nc.sync.dma_start(
                            out_d[ds(tok0 + mt * 128, 128), e, :], o_t)

            # block 0 token DMAs + LN queue before the bulk weight DMAs
            xh_cur = emit_ln(0)
            # weights for expert 0 first (needed soonest), then expert 1
            w1t = [None] * E_LOC
            w2t = [None] * E_LOC
            for e in range(E_LOC):
                w1t[e] = [cst.tile_from(w1_d[e, k], name=f"w1_{e}_{k}")
                          for k in range(KH)]
                w2t[e] = [cst.tile_from(w2_d[e, k], name=f"w2_{e}_{k}")
                          for k in range(KF)]

            for b in range(NBLK):
                xT = emit_transposes(xh_cur)
                if b + 1 < NBLK:
                    xh_cur = emit_ln(b + 1)   # ahead of this block's DVE adds
                emit_experts(b, xT)
    nc.compile()
    return nc


def _get_compiled():
    global _COMPILED
    if _COMPILED is None:
        _COMPILED = _build()
    return _COMPILED


def _prepare_in_maps(tokens, ln_g, ln_b, W1, b1, W2, b2):
    x = np.ascontiguousarray(np.asarray(tokens, dtype=np.float32).reshape(N, H))
    # LN stats (float64 internally; matches fp32 reference to ~1e-7 rel)
    x64 = x.astype(np.float64)
    mu = x64.mean(axis=1)
    var = np.square(x64 - mu[:, None]).mean(axis=1)
    rstd = 1.0 / np.sqrt(var + EPS)
    scale = rstd.astype(np.float32)
    shift = (-mu * rstd).astype(np.float32)

    # Fold LN affine into W1/b1: (x_hat*g + b) @ W1 + b1 = x_hat @ (g*W1) + (b@W1 + b1)
    W1 = np.asarray(W1, dtype=np.float32)
    W2 = np.asarray(W2, dtype=np.float32)
    ln_g = np.asarray(ln_g, dtype=np.float32)
    ln_b = np.asarray(ln_b, dtype=np.float32)
    b1 = np.asarray(b1, dtype=np.float32)
    b2 = np.asarray(b2, dtype=np.float32)
    W1eff = (ln_g[:, :, None] * W1).astype(np.float32)
    b1eff = (np.einsum('eh,ehf->ef', ln_b.astype(np.float64),
                       W1.astype(np.float64)) + b1).astype(np.float32)

    scl = np.ascontiguousarray(scale.reshape(N // 128, 128).T)
    shf = np.ascontiguousarray(shift.reshape(N // 128, 128).T)
    ident = np.eye(128, dtype=np.float32)

    in_maps = []
    for c in range(NCORES):
        e0 = c * E_LOC
        sl = slice(e0, e0 + E_LOC)
        in_maps.append({
            "tokens": x,
            "w1": np.ascontiguousarray(W1eff[sl].reshape(E_LOC, KH, 128, F)),
            "w2": np.ascontiguousarray(W2[sl].reshape(E_LOC, KF, 128, H)),
            "b1c": np.ascontiguousarray(
                b1eff[sl].reshape(E_LOC, MF, 128).transpose(2, 0, 1)
                .reshape(128, E_LOC * MF)),
            "b2bc": np.ascontiguousarray(
                np.broadcast_to(b2[sl][:, None, :], (E_LOC, 128, H))),
            "scl": scl,
            "shf": shf,
            "ident": ident,
        })
    return in_maps


def _run(in_maps, trace=False, **kw):
    nc = _get_compiled()
    return run_bass_kernel_spmd(nc, in_maps, core_ids=list(range(NCORES)),
                                trace=trace, **kw)


def kernel(tokens, ln_g, ln_b, W1, b1, W2, b2):
    in_maps = _prepare_in_maps(tokens, ln_g, ln_b, W1, b1, W2, b2)
    res = _run(in_maps)
    parts = [res.results[c]["out"] for c in range(NCORES)]   # [N, E_LOC, H] each
    full = np.concatenate(parts, axis=1).reshape(B, S, E, H)
    return full.astype(np.float32)
